# revision 2
# baseline (speedup 1.0000x reference)
"""Bilateral grid slicing kernel for Trainium2 (8 NeuronCores, SPMD).

Algorithm per pixel: z = dot(rgb, gray)*; trilinear sample of a
(12, 8, 16, 16) grid at (x, y, z); apply resulting 3x4 affine to rgb.

Distribution: core k handles view k//2, H-half k%2 (540 rows x 1920 cols
= 1,036,800 pixels, padded to 1,048,576 = 64 tiles of 16384).

Per-tile pipeline (T=128 pixels/partition, n=16384):
  1. DMA gxy/rgb -> layout B [128, 128, C]
  2. elementwise prep: ix,iy,iz, floors (with round-fix), fracs, flat idx (f32)
  3. PE-transpose idx -> PSUM [t, p]; 8 scalar-engine drains with bias=corner
     offset -> wrapped int16 idx streams (gather groups = t-subsets)
  4. 8x ap_gather (one per corner) from per-partition channel tables
  5. per corner: 16 r-strided PE-transposes back to pixel-major PSUM,
     multiply by trilinear weight (free-stride-0 broadcast over channels),
     accumulate
  6. affine apply (A[:, :3] @ rgb + A[:, 3]) with AP tricks, DMA out
"""
import sys

sys.path.insert(0, "/opt/trn_rl_repo")
import numpy as np

import concourse.bass as bass
import concourse.bacc as bacc
import concourse.tile as tile
from concourse import mybir
from concourse import bass_utils

F32 = mybir.dt.float32
I16 = mybir.dt.int16
I32 = mybir.dt.int32
Alu = mybir.AluOpType
ActFn = mybir.ActivationFunctionType

N_CORES = 8
H, W = 1080, 1920
HH = H // 2                    # rows per core
P_CORE = HH * W                # 1,036,800 pixels per core
T = 128                        # pixels per partition per tile
N_TILE = 128 * T               # 16384 pixels per tile
N_TILES = (P_CORE + N_TILE - 1) // N_TILE   # 64 (padded)
P_PAD = N_TILES * N_TILE       # 1,048,576

GL, GH, GW = 8, 16, 16
NCELL = GL * GH * GW           # 2048
NCELL_PAD = 2080               # guard for corner offsets at edge cells
# corner offsets in flat cell units, kappa = dz*4 + dy*2 + dx
DELTAS = [dz * 256 + dy * 16 + dx for dz in (0, 1) for dy in (0, 1) for dx in (0, 1)]

_cache = {}


def _build(n_tiles):
    nc = bacc.Bacc("TRN2", target_bir_lowering=False)
    n_pix = n_tiles * N_TILE
    with tile.TileContext(nc) as tc:
        with tc.tile_pool(name="dram", bufs=1, space="DRAM") as dram:
            gxy = dram.tile([n_pix, 2], F32, kind="ExternalInput", name="gxy", uniquify=False)
            rgb = dram.tile([n_pix, 3], F32, kind="ExternalInput", name="rgb", uniquify=False)
            grid = dram.tile([12, NCELL], F32, kind="ExternalInput", name="grid", uniquify=False)
            ident = dram.tile([128, 128], F32, kind="ExternalInput", name="ident", uniquify=False)
            out = dram.tile([n_pix, 3], F32, kind="ExternalOutput", name="out", uniquify=False)
            _body(nc, tc, n_tiles, gxy, rgb, grid, ident, out)
    nc.compile()
    return nc


def _ap(t, extra_dims, offset=0):
    """Custom AP on tile t: keeps the tile's partition pitch, custom free dims."""
    a = t[:] if not isinstance(t, bass.AP) else t
    return bass.AP(tensor=a.tensor, offset=a.offset + offset,
                   ap=[list(a.ap[0])] + [list(d) for d in extra_dims])


def _body(nc, tc, n_tiles, gxy, rgb, grid, ident, out):
    import contextlib
    ctx = contextlib.ExitStack()
    const = ctx.enter_context(tc.tile_pool(name="const", bufs=1))
    io = ctx.enter_context(tc.tile_pool(name="io", bufs=3))
    wk = ctx.enter_context(tc.tile_pool(name="wk", bufs=2))
    gkp = ctx.enter_context(tc.tile_pool(name="gkp", bufs=2))
    accp = ctx.enter_context(tc.tile_pool(name="accp", bufs=2))
    outp = ctx.enter_context(tc.tile_pool(name="outp", bufs=3))
    ps_idx = ctx.enter_context(tc.tile_pool(name="ps_idx", bufs=2, space="PSUM"))
    ps_ta = ctx.enter_context(tc.tile_pool(name="ps_ta", bufs=1, space="PSUM"))

    # --- one-time setup ---------------------------------------------------
    tables = const.tile([128, NCELL_PAD, 1], F32)
    nc.vector.memset(tables[:], 0.0)
    for g in range(8):
        nc.sync.dma_start(out=tables[16 * g:16 * g + 12, 0:NCELL, 0], in_=grid[:, :])
    ident_sb = const.tile([128, 128], F32)
    nc.sync.dma_start(out=ident_sb[:], in_=ident[:])

    for it in range(n_tiles):
        j0 = it * N_TILE
        gxy_t = io.tile([128, T, 2], F32, tag="gxy_t")
        nc.sync.dma_start(out=gxy_t[:], in_=gxy[j0:j0 + N_TILE, :].rearrange("(p t) c -> p t c", p=128))
        rgb_t = io.tile([128, T, 3], F32, tag="rgb_t")
        nc.sync.dma_start(out=rgb_t[:], in_=rgb[j0:j0 + N_TILE, :].rearrange("(p t) c -> p t c", p=128))

        # --- prep: scaled coords ----------------------------------------
        ix = wk.tile([128, T], F32, tag="ix")
        nc.vector.tensor_scalar_mul(ix[:], gxy_t[:, :, 0], float(GW - 1))
        iy = wk.tile([128, T], F32, tag="iy")
        nc.vector.tensor_scalar_mul(iy[:], gxy_t[:, :, 1], float(GH - 1))
        iz = wk.tile([128, T], F32, tag="iz")
        s = float(GL - 1)
        nc.scalar.activation(iz[:], rgb_t[:, :, 0], ActFn.Copy, scale=0.299 * s)
        nc.vector.scalar_tensor_tensor(iz[:], rgb_t[:, :, 1], 0.587 * s, iz[:], Alu.mult, Alu.add)
        nc.vector.scalar_tensor_tensor(iz[:], rgb_t[:, :, 2], 0.114 * s, iz[:], Alu.mult, Alu.add)
        nc.vector.tensor_scalar_min(iz[:], iz[:], 6.9999995)

        # --- floors (robust to round-to-nearest casts) + fracs ----------
        def floorfrac(q, tag):
            qi = wk.tile([128, T], I32, tag=tag + "i")
            nc.vector.tensor_copy(out=qi[:], in_=q[:])
            qf = wk.tile([128, T], F32, tag=tag + "f")
            nc.vector.tensor_copy(out=qf[:], in_=qi[:])
            gt = wk.tile([128, T], F32, tag=tag + "g")
            nc.vector.tensor_tensor(gt[:], qf[:], q[:], Alu.is_gt)
            nc.vector.tensor_tensor(qf[:], qf[:], gt[:], Alu.subtract)
            w = wk.tile([128, T], F32, tag=tag + "w")
            nc.vector.tensor_tensor(w[:], q[:], qf[:], Alu.subtract)
            return qf, w

        x0f, wx = floorfrac(ix, "x")
        y0f, wy = floorfrac(iy, "y")
        z0f, wz = floorfrac(iz, "z")

        idxf = wk.tile([128, T], F32, tag="idxf")
        nc.vector.scalar_tensor_tensor(idxf[:], z0f[:], 16.0, y0f[:], Alu.mult, Alu.add)
        nc.vector.scalar_tensor_tensor(idxf[:], idxf[:], 16.0, x0f[:], Alu.mult, Alu.add)

        # --- trilinear weights w8 [128, T, 8] ---------------------------
        w8 = wk.tile([128, T, 8], F32, tag="w8")
        wx0 = wk.tile([128, T], F32, tag="wx0")
        nc.vector.tensor_scalar(wx0[:], wx[:], -1.0, 1.0, Alu.mult, Alu.add)
        wy0 = wk.tile([128, T], F32, tag="wy0")
        nc.vector.tensor_scalar(wy0[:], wy[:], -1.0, 1.0, Alu.mult, Alu.add)
        wz0 = wk.tile([128, T], F32, tag="wz0")
        nc.vector.tensor_scalar(wz0[:], wz[:], -1.0, 1.0, Alu.mult, Alu.add)
        zy = wk.tile([128, T, 4], F32, tag="zy")
        for a, za in ((0, wz0), (1, wz)):
            for b, yb in ((0, wy0), (1, wy)):
                nc.vector.tensor_tensor(zy[:, :, a * 2 + b], za[:], yb[:], Alu.mult)
        for k in range(8):
            xc = wx0 if (k & 1) == 0 else wx
            nc.vector.tensor_tensor(w8[:, :, k], zy[:, :, k >> 1], xc[:], Alu.mult)

        # --- idx transpose + wrapped int16 streams ----------------------
        pidx = ps_idx.tile([128, 128], F32, tag="pidx")
        nc.tensor.transpose(pidx[:], idxf[:], ident_sb[:])
        wraps = []
        for k in range(8):
            wr = wk.tile([128, 128], I16, tag=f"wr{k}")
            nc.scalar.activation(wr[:], pidx[:], ActFn.Copy, bias=float(DELTAS[k]))
            wraps.append(wr)

        # --- gathers + blend --------------------------------------------
        acc = accp.tile([128, 16 * 8 * 12], F32, tag="acc")   # (r, g, c)
        tmp = accp.tile([128, 16 * 8 * 12], F32, tag="tmp")
        for k in range(8):
            gk = gkp.tile([128, N_TILE // 8, 1], F32, tag="gk")
            nc.gpsimd.ap_gather(gk[:], tables[:], wraps[k][:],
                                channels=128, num_elems=NCELL_PAD, d=1,
                                num_idxs=N_TILE // 8)
            ta = ps_ta.tile([128, 16 * 128], F32, tag="ta")
            for r in range(16):
                nc.tensor.transpose(ta[:, r * 128:(r + 1) * 128], gk[:, r::16, 0], ident_sb[:])
            # dst free layout (r, g, c); ta free layout (r, (g,s16))
            dst = acc if k == 0 else tmp
            nc.vector.tensor_tensor(
                _ap(dst, [[96, 16], [12, 8], [1, 12]]),
                _ap(ta, [[128, 16], [16, 8], [1, 12]]),
                _ap(w8, [[8, 16], [8 * 16, 8], [0, 12]], offset=k),
                Alu.mult)
            if k > 0:
                nc.vector.tensor_tensor(
                    _ap(acc, [[96, 16], [12, 8], [1, 12]]),
                    _ap(acc, [[96, 16], [12, 8], [1, 12]]),
                    _ap(tmp, [[96, 16], [12, 8], [1, 12]]),
                    Alu.add)

        # --- affine apply: out_i = sum_j A[i,j]*u_j + A[i,3] ------------
        # acc free = (r, g, c=(i*4+j));  rgb_t free = (t=(16g+r), ch)
        outA = outp.tile([128, 8 * 16 * 3], F32, tag="outA")  # (g, r, i)
        m2 = accp.tile([128, 16 * 8 * 3], F32, tag="m2")
        red = accp.tile([128, 16 * 8], F32, tag="red")
        for i in range(3):
            nc.vector.tensor_tensor(
                _ap(m2, [[24, 16], [3, 8], [1, 3]]),
                _ap(acc, [[96, 16], [12, 8], [1, 3]], offset=i * 4),
                _ap(rgb_t, [[3, 16], [48, 8], [1, 3]]),
                Alu.mult)
            nc.vector.tensor_reduce(
                red[:].rearrange("p (r g) -> p r g", r=16),
                m2[:].rearrange("p (r g j) -> p r g j", r=16, g=8),
                mybir.AxisListType.X, Alu.add)
            nc.vector.tensor_tensor(
                _ap(outA, [[3, 16], [48, 8]], offset=i),
                _ap(red, [[8, 16], [1, 8]]),
                _ap(acc, [[96, 16], [12, 8]], offset=i * 4 + 3),
                Alu.add)

        # --- DMA out: src (p; g, r*i merged) -> dram (p*T + 16g + r)*3 + i
        nc.sync.dma_start(
            out=bass.AP(tensor=out.tensor, offset=out.offset + j0 * 3,
                        ap=[[T * 3, 128], [48, 8], [1, 48]]),
            in_=_ap(outA, [[48, 8], [1, 48]]))
    ctx.close()


def _shards(grid_xy, rgb, grids):
    """Split full inputs into 8 per-core input maps (padded)."""
    ident = np.eye(128, dtype=np.float32)
    maps = []
    for k in range(N_CORES):
        v, h = k // 2, k % 2
        gxy_s = grid_xy[v, 0, h * HH:(h + 1) * HH].reshape(-1, 2)
        rgb_s = rgb[v, 0, h * HH:(h + 1) * HH].reshape(-1, 3)
        pad = P_PAD - P_CORE
        gxy_s = np.concatenate([gxy_s, np.zeros((pad, 2), np.float32)])
        rgb_s = np.concatenate([rgb_s, np.zeros((pad, 3), np.float32)])
        maps.append({
            "gxy": np.ascontiguousarray(gxy_s),
            "rgb": np.ascontiguousarray(rgb_s),
            "grid": np.ascontiguousarray(grids[v].reshape(12, NCELL)),
            "ident": ident,
        })
    return maps


def kernel(grid_xy, rgb, grids):
    if "nc" not in _cache:
        _cache["nc"] = _build(N_TILES)
    nc = _cache["nc"]
    maps = _shards(grid_xy, rgb, grids)
    res = bass_utils.run_bass_kernel_spmd(nc, maps, core_ids=list(range(N_CORES)))
    outv = np.empty((4, 1, H, W, 3), np.float32)
    for k in range(N_CORES):
        v, h = k // 2, k % 2
        o = res.results[k]["out"][:P_CORE].reshape(HH, W, 3)
        outv[v, 0, h * HH:(h + 1) * HH] = o
    return outv


# revision 3
# speedup vs baseline: 1.4189x; 1.4189x over previous
"""Bilateral grid slicing kernel for Trainium2 (8 NeuronCores, SPMD).

Algorithm per pixel: z = dot(rgb, gray)*; trilinear sample of a
(12, 8, 16, 16) grid at (x, y, z); apply resulting 3x4 affine to rgb.

Distribution: core k handles view k//2, H-half k%2 (540 rows x 1920 cols
= 1,036,800 pixels, padded to 1,048,576 = 64 tiles of 16384).

Per-tile pipeline (T=128 pixels/partition, n=16384):
  1. DMA gxy/rgb -> layout B [128, 128, C]
  2. elementwise prep: ix,iy,iz, floors (with round-fix), fracs, flat idx (f32)
  3. PE-transpose idx -> PSUM [t, p]; 8 scalar-engine drains with bias=corner
     offset -> wrapped int16 idx streams (gather groups = t-subsets)
  4. 8x ap_gather (one per corner) from per-partition channel tables
  5. per corner: 16 r-strided PE-transposes back to pixel-major PSUM,
     multiply by trilinear weight (free-stride-0 broadcast over channels),
     accumulate
  6. affine apply (A[:, :3] @ rgb + A[:, 3]) with AP tricks, DMA out
"""
import sys

sys.path.insert(0, "/opt/trn_rl_repo")
import numpy as np

import concourse.bass as bass
import concourse.bacc as bacc
import concourse.tile as tile
from concourse import mybir
from concourse import bass_utils

F32 = mybir.dt.float32
I16 = mybir.dt.int16
I32 = mybir.dt.int32
Alu = mybir.AluOpType
ActFn = mybir.ActivationFunctionType

N_CORES = 8
H, W = 1080, 1920
HH = H // 2                    # rows per core
P_CORE = HH * W                # 1,036,800 pixels per core
T = 128                        # pixels per partition per tile
N_TILE = 128 * T               # 16384 pixels per tile
N_TILES = (P_CORE + N_TILE - 1) // N_TILE   # 64 (padded)
P_PAD = N_TILES * N_TILE       # 1,048,576

GL, GH, GW = 8, 16, 16
NCELL = GL * GH * GW           # 2048
NCELL_PAD = 2080               # guard for corner offsets at edge cells
# corner offsets in flat cell units, kappa = dz*4 + dy*2 + dx
DELTAS = [dz * 256 + dy * 16 + dx for dz in (0, 1) for dy in (0, 1) for dx in (0, 1)]

_cache = {}


def _build(n_tiles):
    nc = bacc.Bacc("TRN2", target_bir_lowering=False)
    n_pix = n_tiles * N_TILE
    with tile.TileContext(nc) as tc:
        with tc.tile_pool(name="dram", bufs=1, space="DRAM") as dram:
            gxy = dram.tile([n_pix, 2], F32, kind="ExternalInput", name="gxy", uniquify=False)
            rgb = dram.tile([n_pix, 3], F32, kind="ExternalInput", name="rgb", uniquify=False)
            grid = dram.tile([12, NCELL], F32, kind="ExternalInput", name="grid", uniquify=False)
            ident = dram.tile([128, 128], F32, kind="ExternalInput", name="ident", uniquify=False)
            out = dram.tile([n_pix, 3], F32, kind="ExternalOutput", name="out", uniquify=False)
            _body(nc, tc, n_tiles, gxy, rgb, grid, ident, out)
    nc.compile()
    return nc


def _ap(t, extra_dims, offset=0):
    """Custom AP on tile t: keeps the tile's partition pitch, custom free dims."""
    a = t[:] if not isinstance(t, bass.AP) else t
    return bass.AP(tensor=a.tensor, offset=a.offset + offset,
                   ap=[list(a.ap[0])] + [list(d) for d in extra_dims])


def _body(nc, tc, n_tiles, gxy, rgb, grid, ident, out):
    import contextlib
    ctx = contextlib.ExitStack()
    const = ctx.enter_context(tc.tile_pool(name="const", bufs=1))
    io = ctx.enter_context(tc.tile_pool(name="io", bufs=3))
    wk = ctx.enter_context(tc.tile_pool(name="wk", bufs=2))
    gkp = ctx.enter_context(tc.tile_pool(name="gkp", bufs=2))
    accp = ctx.enter_context(tc.tile_pool(name="accp", bufs=2))
    outp = ctx.enter_context(tc.tile_pool(name="outp", bufs=3))
    ps_idx = ctx.enter_context(tc.tile_pool(name="ps_idx", bufs=2, space="PSUM"))
    ps_ta = ctx.enter_context(tc.tile_pool(name="ps_ta", bufs=2, space="PSUM"))

    # --- one-time setup ---------------------------------------------------
    tables = const.tile([128, NCELL_PAD, 1], F32)
    nc.vector.memset(tables[:], 0.0)
    for g in range(8):
        nc.sync.dma_start(out=tables[16 * g:16 * g + 12, 0:NCELL, 0], in_=grid[:, :])
    ident_sb = const.tile([128, 128], F32)
    nc.sync.dma_start(out=ident_sb[:], in_=ident[:])

    for it in range(n_tiles):
        j0 = it * N_TILE
        gxy_t = io.tile([128, T, 2], F32, tag="gxy_t")
        nc.sync.dma_start(out=gxy_t[:], in_=gxy[j0:j0 + N_TILE, :].rearrange("(p t) c -> p t c", p=128))
        rgb_t = io.tile([128, T, 3], F32, tag="rgb_t")
        nc.sync.dma_start(out=rgb_t[:], in_=rgb[j0:j0 + N_TILE, :].rearrange("(p t) c -> p t c", p=128))

        # --- prep: scaled coords ----------------------------------------
        ix = wk.tile([128, T], F32, tag="ix")
        nc.vector.tensor_scalar_mul(ix[:], gxy_t[:, :, 0], float(GW - 1))
        iy = wk.tile([128, T], F32, tag="iy")
        nc.vector.tensor_scalar_mul(iy[:], gxy_t[:, :, 1], float(GH - 1))
        iz = wk.tile([128, T], F32, tag="iz")
        s = float(GL - 1)
        nc.scalar.activation(iz[:], rgb_t[:, :, 0], ActFn.Copy, scale=0.299 * s)
        nc.vector.scalar_tensor_tensor(iz[:], rgb_t[:, :, 1], 0.587 * s, iz[:], Alu.mult, Alu.add)
        nc.vector.scalar_tensor_tensor(iz[:], rgb_t[:, :, 2], 0.114 * s, iz[:], Alu.mult, Alu.add)
        nc.vector.tensor_scalar_min(iz[:], iz[:], 6.9999995)

        # --- floors (robust to round-to-nearest casts) + fracs ----------
        def floorfrac(q, tag):
            qi = wk.tile([128, T], I32, tag=tag + "i")
            nc.vector.tensor_copy(out=qi[:], in_=q[:])
            qf = wk.tile([128, T], F32, tag=tag + "f")
            nc.vector.tensor_copy(out=qf[:], in_=qi[:])
            gt = wk.tile([128, T], F32, tag=tag + "g")
            nc.vector.tensor_tensor(gt[:], qf[:], q[:], Alu.is_gt)
            nc.vector.tensor_tensor(qf[:], qf[:], gt[:], Alu.subtract)
            w = wk.tile([128, T], F32, tag=tag + "w")
            nc.vector.tensor_tensor(w[:], q[:], qf[:], Alu.subtract)
            return qf, w

        x0f, wx = floorfrac(ix, "x")
        y0f, wy = floorfrac(iy, "y")
        z0f, wz = floorfrac(iz, "z")

        idxf = wk.tile([128, T], F32, tag="idxf")
        nc.vector.scalar_tensor_tensor(idxf[:], z0f[:], 16.0, y0f[:], Alu.mult, Alu.add)
        nc.vector.scalar_tensor_tensor(idxf[:], idxf[:], 16.0, x0f[:], Alu.mult, Alu.add)

        # --- trilinear weights w8 [128, T, 8] ---------------------------
        w8 = wk.tile([128, T, 8], F32, tag="w8")
        wx0 = wk.tile([128, T], F32, tag="wx0")
        nc.vector.tensor_scalar(wx0[:], wx[:], -1.0, 1.0, Alu.mult, Alu.add)
        wy0 = wk.tile([128, T], F32, tag="wy0")
        nc.vector.tensor_scalar(wy0[:], wy[:], -1.0, 1.0, Alu.mult, Alu.add)
        wz0 = wk.tile([128, T], F32, tag="wz0")
        nc.vector.tensor_scalar(wz0[:], wz[:], -1.0, 1.0, Alu.mult, Alu.add)
        zy = wk.tile([128, T, 4], F32, tag="zy")
        for a, za in ((0, wz0), (1, wz)):
            for b, yb in ((0, wy0), (1, wy)):
                nc.vector.tensor_tensor(zy[:, :, a * 2 + b], za[:], yb[:], Alu.mult)
        for k in range(8):
            xc = wx0 if (k & 1) == 0 else wx
            nc.vector.tensor_tensor(w8[:, :, k], zy[:, :, k >> 1], xc[:], Alu.mult)

        # --- idx transpose + wrapped int16 streams ----------------------
        pidx = ps_idx.tile([128, 128], F32, tag="pidx")
        nc.tensor.transpose(pidx[:], idxf[:], ident_sb[:])
        wraps = []
        for k in range(8):
            wr = wk.tile([128, 128], I16, tag=f"wr{k}")
            nc.scalar.activation(wr[:], pidx[:], ActFn.Copy, bias=float(DELTAS[k]))
            wraps.append(wr)

        # --- gathers + blend --------------------------------------------
        acc = accp.tile([128, 16 * 8 * 12], F32, tag="acc")   # (r, g, c)
        tmp = accp.tile([128, 16 * 8 * 12], F32, tag="tmp")
        for k in range(8):
            gk = gkp.tile([128, N_TILE // 8, 1], F32, tag="gk")
            nc.gpsimd.ap_gather(gk[:], tables[:], wraps[k][:],
                                channels=128, num_elems=NCELL_PAD, d=1,
                                num_idxs=N_TILE // 8)
            # two PSUM half-buffers (r 0..7, 8..15) so PE transposes of the
            # next half overlap the DVE multiply consuming the previous one
            for half in range(2):
                ta = ps_ta.tile([128, 8 * 128], F32, tag="ta")
                for rr in range(8):
                    r = half * 8 + rr
                    nc.tensor.transpose(ta[:, rr * 128:(rr + 1) * 128], gk[:, r::16, 0], ident_sb[:])
                base = half * 8 * 96      # acc/tmp offset: r stride is 96
                wbase = k + half * 8 * 8  # w8 offset: r stride is 8
                dst = acc if k == 0 else tmp
                nc.vector.tensor_tensor(
                    _ap(dst, [[96, 8], [12, 8], [1, 12]], offset=base),
                    _ap(ta, [[128, 8], [16, 8], [1, 12]]),
                    _ap(w8, [[8, 8], [8 * 16, 8], [0, 12]], offset=wbase),
                    Alu.mult)
                if k > 0:
                    nc.vector.tensor_tensor(
                        _ap(acc, [[96, 8], [12, 8], [1, 12]], offset=base),
                        _ap(acc, [[96, 8], [12, 8], [1, 12]], offset=base),
                        _ap(tmp, [[96, 8], [12, 8], [1, 12]], offset=base),
                        Alu.add)

        # --- affine apply: out_i = sum_j A[i,j]*u_j + A[i,3] ------------
        # acc free = (r, g, c=(i*4+j));  rgb_t free = (t=(16g+r), ch)
        outA = outp.tile([128, 8 * 16 * 3], F32, tag="outA")  # (g, r, i)
        m2 = accp.tile([128, 16 * 8 * 3], F32, tag="m2")
        red = accp.tile([128, 16 * 8], F32, tag="red")
        for i in range(3):
            nc.vector.tensor_tensor(
                _ap(m2, [[24, 16], [3, 8], [1, 3]]),
                _ap(acc, [[96, 16], [12, 8], [1, 3]], offset=i * 4),
                _ap(rgb_t, [[3, 16], [48, 8], [1, 3]]),
                Alu.mult)
            nc.vector.tensor_reduce(
                red[:].rearrange("p (r g) -> p r g", r=16),
                m2[:].rearrange("p (r g j) -> p r g j", r=16, g=8),
                mybir.AxisListType.X, Alu.add)
            nc.vector.tensor_tensor(
                _ap(outA, [[3, 16], [48, 8]], offset=i),
                _ap(red, [[8, 16], [1, 8]]),
                _ap(acc, [[96, 16], [12, 8]], offset=i * 4 + 3),
                Alu.add)

        # --- DMA out: src (p; g, r*i merged) -> dram (p*T + 16g + r)*3 + i
        nc.sync.dma_start(
            out=bass.AP(tensor=out.tensor, offset=out.offset + j0 * 3,
                        ap=[[T * 3, 128], [48, 8], [1, 48]]),
            in_=_ap(outA, [[48, 8], [1, 48]]))
    ctx.close()


def _shards(grid_xy, rgb, grids):
    """Split full inputs into 8 per-core input maps (padded)."""
    ident = np.eye(128, dtype=np.float32)
    maps = []
    for k in range(N_CORES):
        v, h = k // 2, k % 2
        gxy_s = grid_xy[v, 0, h * HH:(h + 1) * HH].reshape(-1, 2)
        rgb_s = rgb[v, 0, h * HH:(h + 1) * HH].reshape(-1, 3)
        pad = P_PAD - P_CORE
        gxy_s = np.concatenate([gxy_s, np.zeros((pad, 2), np.float32)])
        rgb_s = np.concatenate([rgb_s, np.zeros((pad, 3), np.float32)])
        maps.append({
            "gxy": np.ascontiguousarray(gxy_s),
            "rgb": np.ascontiguousarray(rgb_s),
            "grid": np.ascontiguousarray(grids[v].reshape(12, NCELL)),
            "ident": ident,
        })
    return maps


def kernel(grid_xy, rgb, grids):
    if "nc" not in _cache:
        _cache["nc"] = _build(N_TILES)
    nc = _cache["nc"]
    maps = _shards(grid_xy, rgb, grids)
    res = bass_utils.run_bass_kernel_spmd(nc, maps, core_ids=list(range(N_CORES)))
    outv = np.empty((4, 1, H, W, 3), np.float32)
    for k in range(N_CORES):
        v, h = k // 2, k % 2
        o = res.results[k]["out"][:P_CORE].reshape(HH, W, 3)
        outv[v, 0, h * HH:(h + 1) * HH] = o
    return outv
